# revision 1
# baseline (speedup 1.0000x reference)
"""Trainium2 Bass kernel for nn_AttnBlock: dynamic-filter correlation.

Math (per sample b):
  p1[l, :]  = 11x11x64 patch of im1 at position l (l over 30x30)
  scores[p, l] = <im2 patch at p, p1[l] / max(||p1[l]||, 1e-4)>
  out[p] = max_l scores[p, l]

Decomposition on device (per core = one (sample, p-half) pair):
  scores_un[p, l] = sum_{dy,dx} sum_c im2[c, p+(dy,dx)] * im1[c, l+(dy,dx)]
computed as 121 shift-matmuls (contraction over channels) accumulated in
PSUM, two shifts packed per matmul (K=128).  The shift-compacted bf16
operand tiles are built on the HOST (pure layout + dtype staging, zero
FLOPs) and DMA'd in directly: partitions 0..63 hold the dx-compacted
image, partitions 64..127 the same shifted one more column (or one row
for the dx=10 tile), so each K=128 matmul covers two (dy,dx) shifts.
The moving-side im1 tiles are split per l-half (rows 0..24 / 15..39) so
the first half's DMA bytes are 40% smaller; the DMA kick order is laid
out against the matmul stream's per-tile consumption deadlines.
Output positions p are chunked flat as {128,128,128,66} so the
stationary operand is a full-128-column 16-bit weight load (enables
fast weight load; LDWEIGHTS ~100ns hides under the 450-column stream,
giving ~189ns/matmul back-to-back at the warm 2.4 GHz clock; the chip
sometimes runs the whole kernel at the P0 2.0 GHz power state, which
scales the matmul stream by 1.2x run-to-run).

Norms: separable 11x11 box sum of im1^2 as an fp16 shift-add log tree
on DVE (2x throughput vs fp32; the box sums are ~7.7k so fp16 rounding
is ~1e-3 relative) on a row-split [128, 25, 40] layout covering both
l-halves at once, channel sum via one f16 ones-matmul per half, then
max -> sqrt (scalar ACT) -> DVE reciprocal -> f16 rank-1 broadcast
matmul to [128, 900].  (tensor_tensor_reduce and the custom-DVE
reciprocal_approx_fast both fault this runtime — NRT_EXEC_UNIT_
UNRECOVERABLE — so the epilogue uses plain mult/reduce/max and the
norm chain the plain reciprocal, placed off the critical path.)

Per-chunk epilogues are split per l-half so the j0 half runs on DVE
while the PE streams j1, leaving only ~2us of epilogue after the last
matmul.  A short burst of dummy matmuls during the input-DMA wait trips
the PE HAM activity window so the real matmuls start at 2.4 GHz.

Sharding: 8 cores = 4 samples x 2 halves of the output-row dim (pure
data parallel, no cross-core communication).
"""

import sys

import numpy as np
import ml_dtypes

if "/opt/trn_rl_repo" not in sys.path:
    sys.path.insert(0, "/opt/trn_rl_repo")

B = 4
C = 64
H = W = 40
KER = 11
HP = WP = H - KER + 1  # 30
HALF = HP // 2  # 15 output rows per core
N_CORES = 2 * B
IM2_ROWS = HALF + KER - 1  # 25 input rows needed per half

NL = HALF * WP  # 450 l-columns per half
DX_BASES = [0, 2, 4, 6, 8, 10]
# flat output-position chunks (M = stationary free dim / PSUM partitions)
P_CHUNKS = [(0, 128), (128, 128), (256, 128), (384, 66)]
N_WARMUP = 6

_PROGRAM = None


def _build_program():
    import concourse.bass as bass
    import concourse.tile as tile
    from concourse import bacc

    mybir = bass.mybir
    dt = mybir.dt
    f32 = dt.float32
    f16 = dt.float16
    bf16 = dt.bfloat16
    from contextlib import ExitStack

    nc = bacc.Bacc(
        "TRN2",
        target_bir_lowering=False,
        debug=False,
        enable_asserts=False,
        num_devices=N_CORES,
    )
    im2c_d = [
        nc.dram_tensor(f"im2c{bi}", [128, IM2_ROWS * WP], bf16, kind="ExternalInput").ap()
        for bi in range(6)
    ]
    im1a_d = [
        nc.dram_tensor(f"im1a{bi}", [128, IM2_ROWS * WP], bf16, kind="ExternalInput").ap()
        for bi in range(6)
    ]
    im1b_d = [
        nc.dram_tensor(f"im1b{bi}", [128, IM2_ROWS * WP], bf16, kind="ExternalInput").ap()
        for bi in range(6)
    ]
    im1n_d = nc.dram_tensor("im1n", [128, IM2_ROWS, W], bf16, kind="ExternalInput").ap()
    out_d = nc.dram_tensor("out", [128, 4], f32, kind="ExternalOutput").ap()

    MULT = mybir.AluOpType.mult
    MAX = mybir.AluOpType.max
    SQRT = mybir.ActivationFunctionType.Sqrt

    with tile.TileContext(nc) as tc, ExitStack() as ctx:
        consts = ctx.enter_context(tc.tile_pool(name="consts", bufs=1))
        imgs = ctx.enter_context(tc.tile_pool(name="imgs", bufs=1))
        nrm = ctx.enter_context(tc.tile_pool(name="nrm", bufs=1))
        scr = ctx.enter_context(tc.tile_pool(name="scr", bufs=2))
        reds = ctx.enter_context(tc.tile_pool(name="reds", bufs=8))
        psum = ctx.enter_context(tc.tile_pool(name="psum", bufs=8, space="PSUM"))

        im2c = [imgs.tile([128, IM2_ROWS * WP], bf16, name=f"im2c_{bi}") for bi in range(6)]
        im1a = [imgs.tile([128, IM2_ROWS * WP], bf16, name=f"im1a_{bi}") for bi in range(6)]
        im1b = [imgs.tile([128, IM2_ROWS * WP], bf16, name=f"im1b_{bi}") for bi in range(6)]
        im1n = imgs.tile([128, IM2_ROWS, W], bf16)

        # Warm-up consts on the gpsimd queue, which exits the framework
        # preamble earliest, so the PE warm-up matmuls start ASAP.  The
        # warm-up matmuls are FULL-ARRAY (K=128, M=128): the HAM watches
        # array activity, and the earlier K=1 warm-ups never registered
        # (measured warm transitions only counted from the first real
        # matmul).
        warm_lhs = consts.tile([128, 128], f16)
        nc.gpsimd.memset(warm_lhs[:], 1.0)
        warm_rhs = consts.tile([128, 512], f16)
        nc.gpsimd.memset(warm_rhs[:], 0.25)
        ones_row = consts.tile([1, 128], f16)
        nc.vector.memset(ones_row[:], 1.0)
        ones_col = consts.tile([128, 1], f16)
        nc.vector.memset(ones_col[:], 1.0)
        red_all = reds.tile([128, 4], f32, name="red_all")
        nc.vector.memset(red_all[:], 0.0)

        # DMA kicks, ordered against consumption deadlines: the j0 stream
        # consumes (im2c[k], im1a[k]) at ~10.7 + 2.1k us; im1b[k] only at
        # ~23.5 + 2.1k us; im1n feeds the norm tree whose result is needed
        # by the PE at ~25us.
        nc.sync.dma_start(im2c[0][:], im2c_d[0])
        nc.scalar.dma_start(im1a[0][:], im1a_d[0])
        nc.gpsimd.dma_start(im2c[1][:], im2c_d[1])
        nc.sync.dma_start(im1a[1][:], im1a_d[1])
        nc.scalar.dma_start(im2c[2][:], im2c_d[2])
        nc.gpsimd.dma_start(im1a[2][:], im1a_d[2])
        nc.sync.dma_start(im1a[3][:], im1a_d[3])
        nc.scalar.dma_start(im1n[:], im1n_d)
        nc.gpsimd.dma_start(im2c[3][:], im2c_d[3])
        nc.sync.dma_start(im2c[4][:], im2c_d[4])
        nc.scalar.dma_start(im1a[4][:], im1a_d[4])
        nc.gpsimd.dma_start(im1a[5][:], im1a_d[5])
        nc.sync.dma_start(im1b[0][:], im1b_d[0])
        nc.scalar.dma_start(im2c[5][:], im2c_d[5])
        nc.gpsimd.dma_start(im1b[1][:], im1b_d[1])
        nc.sync.dma_start(im1b[3][:], im1b_d[3])
        nc.scalar.dma_start(im1b[2][:], im1b_d[2])
        nc.gpsimd.dma_start(im1b[4][:], im1b_d[4])
        nc.sync.dma_start(im1b[5][:], im1b_d[5])

        # ---- PE warm-up: trip the HAM activity window during the DMA wait
        # so the real matmuls start at 2.4 GHz.  Results are never read.
        wps = psum.tile([128, 512], f32, tag="ps", name="warm")
        for i in range(N_WARMUP):
            nc.tensor.matmul(wps[:], warm_lhs[:], warm_rhs[:], start=True, stop=True)

        # ---- norm DVE chain: separable 11x11 box sum of im1^2 over (y, x)
        # on the row-split layout (partitions 0..63 = rows 0..24 -> l-half 0,
        # partitions 64..127 = rows 15..39 -> l-half 1), all in fp16.
        sq = nrm.tile([128, IM2_ROWS, W], f16)
        nc.vector.tensor_tensor(out=sq[:], in0=im1n[:], in1=im1n[:], op=MULT)
        t2 = nrm.tile([128, IM2_ROWS, W - 1], f16)
        nc.vector.tensor_add(t2[:], sq[:, :, 0 : W - 1], sq[:, :, 1:W])
        t4 = nrm.tile([128, IM2_ROWS, W - 3], f16)
        nc.vector.tensor_add(t4[:], t2[:, :, 0 : W - 3], t2[:, :, 2 : W - 1])
        t8 = nrm.tile([128, IM2_ROWS, W - 7], f16)
        nc.vector.tensor_add(t8[:], t4[:, :, 0 : W - 7], t4[:, :, 4 : W - 3])
        rpa = nrm.tile([128, IM2_ROWS, WP], f16)
        nc.vector.tensor_add(rpa[:], t8[:, :, 0:WP], t2[:, :, 8 : 8 + WP])
        rp = nrm.tile([128, IM2_ROWS, WP], f16)
        nc.vector.tensor_add(rp[:], rpa[:], sq[:, :, 10 : 10 + WP])

        u2 = nrm.tile([128, IM2_ROWS - 1, WP], f16)
        nc.vector.tensor_add(u2[:], rp[:, 0 : IM2_ROWS - 1], rp[:, 1:IM2_ROWS])
        u4 = nrm.tile([128, IM2_ROWS - 3, WP], f16)
        nc.vector.tensor_add(u4[:], u2[:, 0 : IM2_ROWS - 3], u2[:, 2 : IM2_ROWS - 1])
        u8 = nrm.tile([128, IM2_ROWS - 7, WP], f16)
        nc.vector.tensor_add(u8[:], u4[:, 0 : IM2_ROWS - 7], u4[:, 4 : IM2_ROWS - 3])
        nca = nrm.tile([128, HALF, WP], f16)
        nc.vector.tensor_add(nca[:], u8[:, 0:HALF], u2[:, 8 : 8 + HALF])
        normc = nrm.tile([128, HALF, WP], f16)
        nc.vector.tensor_add(normc[:], nca[:], rp[:, 10 : 10 + HALF])
        normc_f = normc[:].rearrange("p y x -> p (y x)")

        # ---- main correlation matmuls.  121 shifts = 60 packed pairs + 1
        # K=64 single (dy=10, dx=10).  Flat-p chunks, M=128 stationary.
        def emit_chunk_j(p0, M, j, ps_j):
            src = im1a if j == 0 else im1b
            first = True
            for bi, dx in enumerate(DX_BASES):
                dys = range(KER) if dx < 10 else range(0, KER, 2)
                for dy in dys:
                    kp = C if (dx == 10 and dy == 10) else 2 * C
                    lhsT = im2c[bi][0:kp, p0 + WP * dy : p0 + WP * dy + M]
                    rhs = src[bi][0:kp, dy * WP : dy * WP + NL]
                    last = dx == 10 and dy == 10
                    nc.tensor.matmul(ps_j[0:M], lhsT, rhs, start=first, stop=last)
                    first = False

        def alloc_ps(ci):
            return [
                psum.tile([128, NL], f32, tag="ps", name=f"ps_{ci}_{j}")
                for j in range(2)
            ]

        def emit_epi_j0(ci, M, ps):
            sc0 = scr.tile([128, NL], f32, tag="sc", name=f"sc0_{ci}")
            red0 = reds.tile([128, 1], f32, tag="red", name=f"red0_{ci}")
            nc.vector.tensor_tensor(
                out=sc0[0:M], in0=ps[0][0:M], in1=inv_bc[0:M, 0:NL], op=MULT
            )
            nc.vector.tensor_reduce(
                out=red0[0:M], in_=sc0[0:M], axis=mybir.AxisListType.X, op=MAX
            )
            return red0

        def emit_epi_j1(ci, M, ps, red0):
            sc1 = scr.tile([128, NL], f32, tag="sc", name=f"sc1_{ci}")
            red1 = reds.tile([128, 1], f32, tag="red2", name=f"red1_{ci}")
            nc.vector.tensor_tensor(
                out=sc1[0:M], in0=ps[1][0:M], in1=inv_bc[0:M, NL : 2 * NL], op=MULT
            )
            nc.vector.tensor_reduce(
                out=red1[0:M], in_=sc1[0:M], axis=mybir.AxisListType.X, op=MAX
            )
            nc.vector.tensor_tensor(
                out=red_all[0:M, ci : ci + 1], in0=red0[0:M], in1=red1[0:M], op=MAX
            )

        chunk_ps = {}
        chunk_red = {}

        # chunk 0, both l-halves
        chunk_ps[0] = alloc_ps(0)
        emit_chunk_j(*P_CHUNKS[0], 0, chunk_ps[0][0])
        emit_chunk_j(*P_CHUNKS[0], 1, chunk_ps[0][1])

        # norm matmuls: f16 ones channel-sum per l-half, placed after chunk
        # 0 (~34us in) so the fp16 tree has a wide completion margin.
        nm = [psum.tile([1, NL], f32, tag="ps", name=f"nm_{j}") for j in range(2)]
        nc.tensor.matmul(nm[0][:], ones_col[0:C, :], normc_f[0:C, :], start=True, stop=True)
        nc.tensor.matmul(nm[1][:], ones_col[C : 2 * C, :], normc_f[C : 2 * C, :], start=True, stop=True)

        nsq = nrm.tile([1, 2 * NL], f32)
        nc.vector.tensor_scalar_max(nsq[:, 0:NL], nm[0][:], 1e-8)
        nc.vector.tensor_scalar_max(nsq[:, NL : 2 * NL], nm[1][:], 1e-8)
        nrm_s = nrm.tile([1, 2 * NL], f32)
        nc.scalar.activation(nrm_s[:], nsq[:], SQRT)
        inv_s = nrm.tile([1, 2 * NL], f32)
        nc.vector.reciprocal(inv_s[:], nrm_s[:])
        inv16 = nrm.tile([1, 2 * NL], f16)
        nc.vector.tensor_copy(inv16[:], inv_s[:])

        # chunk 1 first half
        chunk_ps[1] = alloc_ps(1)
        emit_chunk_j(*P_CHUNKS[1], 0, chunk_ps[1][0])

        # rank-1 broadcast of 1/norm to all 128 partitions, between chunk
        # 1's halves (inv16 is ready by the time the PE arrives here).
        inv_bc = nrm.tile([128, 2 * NL], f32)
        for j in range(2):
            ip = psum.tile([128, NL], f32, tag="ps", name=f"ip_{j}")
            nc.tensor.matmul(ip[:], ones_row[:], inv16[:, NL * j : NL * (j + 1)], start=True, stop=True)
            nc.vector.tensor_copy(inv_bc[:, NL * j : NL * (j + 1)], ip[:])

        emit_chunk_j(*P_CHUNKS[1], 1, chunk_ps[1][1])

        chunk_red[0] = emit_epi_j0(0, P_CHUNKS[0][1], chunk_ps[0])
        chunk_ps[2] = alloc_ps(2)
        emit_chunk_j(*P_CHUNKS[2], 0, chunk_ps[2][0])
        emit_epi_j1(0, P_CHUNKS[0][1], chunk_ps[0], chunk_red[0])
        nc.gpsimd.dma_start(out_d[:, 0:1], red_all[:, 0:1])
        chunk_red[1] = emit_epi_j0(1, P_CHUNKS[1][1], chunk_ps[1])
        emit_chunk_j(*P_CHUNKS[2], 1, chunk_ps[2][1])
        emit_epi_j1(1, P_CHUNKS[1][1], chunk_ps[1], chunk_red[1])
        nc.gpsimd.dma_start(out_d[:, 1:2], red_all[:, 1:2])
        chunk_ps[3] = alloc_ps(3)
        emit_chunk_j(*P_CHUNKS[3], 0, chunk_ps[3][0])
        chunk_red[2] = emit_epi_j0(2, P_CHUNKS[2][1], chunk_ps[2])
        emit_chunk_j(*P_CHUNKS[3], 1, chunk_ps[3][1])
        emit_epi_j1(2, P_CHUNKS[2][1], chunk_ps[2], chunk_red[2])
        nc.gpsimd.dma_start(out_d[:, 2:3], red_all[:, 2:3])
        chunk_red[3] = emit_epi_j0(3, P_CHUNKS[3][1], chunk_ps[3])
        emit_epi_j1(3, P_CHUNKS[3][1], chunk_ps[3], chunk_red[3])
        nc.gpsimd.dma_start(out_d[:, 3:4], red_all[:, 3:4])

    nc.compile()
    return nc


def _get_program():
    global _PROGRAM
    if _PROGRAM is None:
        _PROGRAM = _build_program()
    return _PROGRAM


def _stack_shift(lo, hi):
    """[64, R, 30] + [64, R, 30] -> [128, R*30] f16."""
    out = np.concatenate([lo, hi], axis=0)
    return np.ascontiguousarray(out.reshape(128, -1).astype(ml_dtypes.bfloat16))


def make_in_maps(im1: np.ndarray, im2: np.ndarray):
    im1 = np.asarray(im1, dtype=np.float32)
    im2 = np.asarray(im2, dtype=np.float32)
    in_maps = []
    for b in range(B):
        i1 = im1[b]
        i1pad = np.concatenate([i1, np.zeros((C, 1, W), np.float32)], axis=1)
        im1_tiles = {}
        for bi, dx in enumerate(DX_BASES):
            for half, key in ((0, f"im1a{bi}"), (1, f"im1b{bi}")):
                y0 = HALF * half
                r = i1[:, y0 : y0 + IM2_ROWS, :]
                rs = i1pad[:, y0 + 1 : y0 + 1 + IM2_ROWS, :]
                if dx < 10:
                    im1_tiles[key] = _stack_shift(
                        r[:, :, dx : dx + WP], r[:, :, dx + 1 : dx + WP + 1]
                    )
                else:
                    im1_tiles[key] = _stack_shift(r[:, :, 10:40], rs[:, :, 10:40])
        im1n = np.ascontiguousarray(
            np.concatenate(
                [i1[:, 0:IM2_ROWS, :], i1[:, HALF : HALF + IM2_ROWS, :]], axis=0
            ).astype(ml_dtypes.bfloat16)
        )
        for h in range(2):
            y0 = HALF * h
            i2 = im2[b][:, y0 : y0 + IM2_ROWS, :]
            i2pad = np.concatenate(
                [im2[b], np.zeros((C, 1, W), np.float32)], axis=1
            )[:, y0 + 1 : y0 + 1 + IM2_ROWS, :]
            m = dict(im1_tiles)
            m["im1n"] = im1n
            for bi, dx in enumerate(DX_BASES):
                if dx < 10:
                    m[f"im2c{bi}"] = _stack_shift(
                        i2[:, :, dx : dx + WP], i2[:, :, dx + 1 : dx + WP + 1]
                    )
                else:
                    m[f"im2c{bi}"] = _stack_shift(i2[:, :, 10:40], i2pad[:, :, 10:40])
            in_maps.append(m)
    return in_maps


def _half_from_cols(cols):
    flat = np.empty((HALF * WP,), dtype=np.float32)
    for ci, (p0, M) in enumerate(P_CHUNKS):
        flat[p0 : p0 + M] = cols[0:M, ci]
    return flat.reshape(HALF, WP)


def assemble(results):
    out = np.empty((B, 1, HP, WP), dtype=np.float32)
    for b in range(B):
        top = _half_from_cols(results[2 * b]["out"])
        bot = _half_from_cols(results[2 * b + 1]["out"])
        out[b, 0] = np.concatenate([top, bot], axis=0)
    return out


def run(im1: np.ndarray, im2: np.ndarray, trace: bool = False):
    from concourse import bass_utils

    nc = _get_program()
    res = bass_utils.run_bass_kernel_spmd(
        nc, make_in_maps(im1, im2), core_ids=list(range(N_CORES)), trace=trace
    )
    return assemble(res.results), res


def kernel(im1: np.ndarray, im2: np.ndarray) -> np.ndarray:
    out, _ = run(np.asarray(im1), np.asarray(im2))
    return out



# revision 12
# speedup vs baseline: 1.1960x; 1.1960x over previous
"""Trainium2 Bass kernel for nn_AttnBlock: dynamic-filter correlation.

Math (per sample b):
  p1[l, :]  = 11x11x64 patch of im1 at position l (l over 30x30)
  scores[p, l] = <im2 patch at p, p1[l] / max(||p1[l]||, 1e-4)>
  out[p] = max_l scores[p, l]

Decomposition on device (per core = one (sample, p-half) pair):
  scores_un[p, l] = sum_{dy,dx} sum_c im2[c, p+(dy,dx)] * im1[c, l+(dy,dx)]
computed as 121 shift-matmuls (contraction over channels) accumulated in
PSUM.  Shift-compacted operand tiles are built on the HOST (layout +
dtype staging, zero FLOPs): partitions 0..63 hold the dx-compacted
image, partitions 64..127 the same shifted one more column (or row for
the dx=10 tile), so each K=128 plane covers two (dy,dx) shifts.

Mixed precision: 30 of the 60.5 planes (dx 0..3 all dy; dx 4,5 dy<8)
are fp8 e4m3 packed TWO planes per matmul with DoubleRow perf mode (2x
the bf16 FLOP rate; measured 190ns per 450-col matmul either way), the
rest stay bf16.  This lands rel_err ~1.6e-2 against the 2e-2 gate
(measured: all-bf16 1.5e-3, all-fp8 2.2e-2).  The fp8 stationary tiles
use a padded row pitch of 32 so every DoubleRow plane-pair stride is a
multiple of 16 bytes (s3_lw_dual_fp8 ISA restriction); output positions
are enumerated in this padded space everywhere (the bf16 stationary
tiles are padded the same way; dead px=30,31 columns are zero and
dropped on assembly).

Norms: separable 11x11 box sum of im1^2 as an fp16 shift-add log tree
on DVE on a row-split [128, 25, 40] layout covering both l-halves at
once, channel sum via one f16 ones-matmul per half, then max -> Rsqrt
(scalar ACT) -> f16 rank-1 broadcast matmul to [128, 900].

Per-chunk epilogues are split per l-half so the j0 half runs on DVE
while the PE streams j1.  A burst of dummy matmuls during the
input-DMA wait trips the PE HAM activity window so the real matmuls
start at 2.4 GHz.  Input DMA is sliced and kicked in consumption-
deadline order (fp8 bi0 tiles first) so the stream never stalls.

Sharding: 8 cores = 4 samples x 2 halves of the output-row dim (pure
data parallel, no cross-core communication).
"""

import sys

import numpy as np
import ml_dtypes

if "/opt/trn_rl_repo" not in sys.path:
    sys.path.insert(0, "/opt/trn_rl_repo")

B = 4
C = 64
H = W = 40
KER = 11
HP = WP = H - KER + 1  # 30
HALF = HP // 2  # 15 output rows per core
N_CORES = 2 * B
IM2_ROWS = HALF + KER - 1  # 25 input rows needed per half

NL = HALF * WP  # 450 l-columns per half
DX_BASES = [0, 2, 4, 6, 8, 10]
TILE_F = IM2_ROWS * WP  # 750 free elements per dx-base moving tile
WPAD = 32
TILE_FW = IM2_ROWS * WPAD  # 800 free elements per padded stationary tile
# padded output-position chunks (M = stationary free dim / PSUM partitions)
P_CHUNKS = [(0, 128), (128, 128), (256, 128), (384, 96)]
N_WARMUP = 8

# fp8 plane set S: bi 0,1 all dy; bi 2 dy 0..7  (30 planes = 60 shifts).
# DoubleRow pairs within the fp8 tensors ([bi0,bi1,bi2] concatenated):
# (stationary offset, stat stride, moving offset, mov stride).
FP8_PAIRS = (
    [
        (TILE_FW * bi + WPAD * dy, WPAD, TILE_F * bi + WP * dy, WP)
        for bi in range(2)
        for dy in (0, 2, 4, 6, 8)
    ]
    + [(WPAD * 10, TILE_FW, WP * 10, TILE_F)]  # (0,10)+(1,10)
    + [
        (TILE_FW * 2 + WPAD * dy, WPAD, TILE_F * 2 + WP * dy, WP)
        for dy in (0, 2, 4, 6)
    ]
)
# bf16 planes (within the bf16 tensors, [bi2,bi3,bi4,bi5] concatenated):
# (stationary offset, moving offset, K)
BF16_PLANES = (
    [(WPAD * dy, WP * dy, 128) for dy in (8, 9, 10)]  # bi2 remainder
    + [(TILE_FW + WPAD * dy, TILE_F + WP * dy, 128) for dy in range(11)]  # bi3
    + [(2 * TILE_FW + WPAD * dy, 2 * TILE_F + WP * dy, 128) for dy in range(11)]  # bi4
    + [(3 * TILE_FW + WPAD * dy, 3 * TILE_F + WP * dy, 128) for dy in (0, 2, 4, 6, 8)]
    + [(3 * TILE_FW + WPAD * 10, 3 * TILE_F + WP * 10, C)]  # (10,10), K=64, last
)

_PROGRAM = None


def _build_program():
    import concourse.bass as bass
    import concourse.tile as tile
    from concourse import bacc
    from concourse.ap import AP

    mybir = bass.mybir
    dt = mybir.dt
    f32 = dt.float32
    f16 = dt.float16
    bf16 = dt.bfloat16
    f8 = dt.float8e4
    DR = mybir.MatmulPerfMode.DoubleRow
    from contextlib import ExitStack

    nc = bacc.Bacc(
        "TRN2",
        target_bir_lowering=False,
        debug=False,
        enable_asserts=False,
        num_devices=N_CORES,
    )
    w8_d = nc.dram_tensor("w8", [128, 3 * TILE_FW], f8, kind="ExternalInput").ap()
    a8_d = nc.dram_tensor("a8", [128, 3 * TILE_F], f8, kind="ExternalInput").ap()
    b8_d = nc.dram_tensor("b8", [128, 3 * TILE_F], f8, kind="ExternalInput").ap()
    w16_d = nc.dram_tensor("w16", [128, 4 * TILE_FW], bf16, kind="ExternalInput").ap()
    a16_d = nc.dram_tensor("a16", [128, 4 * TILE_F], bf16, kind="ExternalInput").ap()
    b16_d = nc.dram_tensor("b16", [128, 4 * TILE_F], bf16, kind="ExternalInput").ap()
    im1n_d = nc.dram_tensor("im1n", [128, IM2_ROWS, W], bf16, kind="ExternalInput").ap()
    out_d = nc.dram_tensor("out", [128, 4], f32, kind="ExternalOutput").ap()

    MULT = mybir.AluOpType.mult
    MAX = mybir.AluOpType.max
    SQRT = mybir.ActivationFunctionType.Sqrt

    with tile.TileContext(nc) as tc, ExitStack() as ctx:
        consts = ctx.enter_context(tc.tile_pool(name="consts", bufs=1))
        imgs = ctx.enter_context(tc.tile_pool(name="imgs", bufs=1))
        nrm = ctx.enter_context(tc.tile_pool(name="nrm", bufs=1))
        scr = ctx.enter_context(tc.tile_pool(name="scr", bufs=2))
        reds = ctx.enter_context(tc.tile_pool(name="reds", bufs=8))
        psum = ctx.enter_context(tc.tile_pool(name="psum", bufs=8, space="PSUM"))

        w8 = imgs.tile([128, 3 * TILE_FW], f8, name="w8")
        a8 = imgs.tile([128, 3 * TILE_F], f8, name="a8")
        b8 = imgs.tile([128, 3 * TILE_F], f8, name="b8")
        w16 = imgs.tile([128, 4 * TILE_FW], bf16, name="w16")
        a16 = imgs.tile([128, 4 * TILE_F], bf16, name="a16")
        b16 = imgs.tile([128, 4 * TILE_F], bf16, name="b16")
        im1n = imgs.tile([128, IM2_ROWS, W], bf16)

        def pair_ap(t, off, step, length, kp=128):
            base = t[:]
            pstride = base.ap[0][0]
            return AP(base.tensor, base.offset + off, [[pstride, kp], [step, 2], [1, length]])

        # Warm-up consts on vector (no DMA kicks there), so the PE warm-up
        # matmuls can start as soon as the framework preamble retires.
        warm_lhs = consts.tile([128, 128], f16)
        nc.vector.memset(warm_lhs[:], 1.0)
        warm_rhs = consts.tile([128, 512], f16)
        nc.vector.memset(warm_rhs[:], 0.25)
        ones_row = consts.tile([1, 128], f16)
        nc.vector.memset(ones_row[:], 1.0)
        ones_col = consts.tile([128, 1], f16)
        nc.vector.memset(ones_col[:], 1.0)
        red_all = reds.tile([128, 4], f32, name="red_all")
        nc.vector.memset(red_all[:], 0.0)

        # DMA kicks in consumption-deadline order: the j0 stream consumes
        # (w8, a8) bi0 first, then bi1,2, then the bf16 tiles; b8/b16 only
        # at the first j1 stream; im1n feeds the norm tree.
        nc.sync.dma_start(w8[:, 0:TILE_FW], w8_d[:, 0:TILE_FW])
        nc.scalar.dma_start(a8[:, 0:TILE_F], a8_d[:, 0:TILE_F])
        nc.gpsimd.dma_start(w8[:, TILE_FW : 3 * TILE_FW], w8_d[:, TILE_FW : 3 * TILE_FW])
        nc.sync.dma_start(a8[:, TILE_F : 3 * TILE_F], a8_d[:, TILE_F : 3 * TILE_F])
        nc.scalar.dma_start(w16[:], w16_d)
        nc.gpsimd.dma_start(a16[:], a16_d)
        nc.sync.dma_start(im1n[:], im1n_d)
        nc.scalar.dma_start(b8[:], b8_d)
        nc.gpsimd.dma_start(b16[:], b16_d)

        # ---- PE warm-up: trip the HAM activity window during the DMA wait
        # so the real matmuls start at 2.4 GHz.  Results are never read.
        wps = psum.tile([128, 512], f32, tag="ps", name="warm")
        for i in range(N_WARMUP):
            nc.tensor.matmul(wps[:], warm_lhs[:], warm_rhs[:], start=True, stop=True)

        # ---- norm DVE chain: separable 11x11 box sum of im1^2 over (y, x)
        # on the row-split layout (partitions 0..63 = rows 0..24 -> l-half 0,
        # partitions 64..127 = rows 15..39 -> l-half 1), all in fp16.
        sq = nrm.tile([128, IM2_ROWS, W], f16)
        nc.vector.tensor_tensor(out=sq[:], in0=im1n[:], in1=im1n[:], op=MULT)
        t2 = nrm.tile([128, IM2_ROWS, W - 1], f16)
        nc.vector.tensor_add(t2[:], sq[:, :, 0 : W - 1], sq[:, :, 1:W])
        t4 = nrm.tile([128, IM2_ROWS, W - 3], f16)
        nc.vector.tensor_add(t4[:], t2[:, :, 0 : W - 3], t2[:, :, 2 : W - 1])
        t8 = nrm.tile([128, IM2_ROWS, W - 7], f16)
        nc.vector.tensor_add(t8[:], t4[:, :, 0 : W - 7], t4[:, :, 4 : W - 3])
        rpa = nrm.tile([128, IM2_ROWS, WP], f16)
        nc.vector.tensor_add(rpa[:], t8[:, :, 0:WP], t2[:, :, 8 : 8 + WP])
        rp = nrm.tile([128, IM2_ROWS, WP], f16)
        nc.vector.tensor_add(rp[:], rpa[:], sq[:, :, 10 : 10 + WP])

        u2 = nrm.tile([128, IM2_ROWS - 1, WP], f16)
        nc.vector.tensor_add(u2[:], rp[:, 0 : IM2_ROWS - 1], rp[:, 1:IM2_ROWS])
        u4 = nrm.tile([128, IM2_ROWS - 3, WP], f16)
        nc.vector.tensor_add(u4[:], u2[:, 0 : IM2_ROWS - 3], u2[:, 2 : IM2_ROWS - 1])
        u8 = nrm.tile([128, IM2_ROWS - 7, WP], f16)
        nc.vector.tensor_add(u8[:], u4[:, 0 : IM2_ROWS - 7], u4[:, 4 : IM2_ROWS - 3])
        nca = nrm.tile([128, HALF, WP], f16)
        nc.vector.tensor_add(nca[:], u8[:, 0:HALF], u2[:, 8 : 8 + HALF])
        normc = nrm.tile([128, HALF, WP], f16)
        nc.vector.tensor_add(normc[:], nca[:], rp[:, 10 : 10 + HALF])
        normc_f = normc[:].rearrange("p y x -> p (y x)")

        # ---- main correlation matmuls: 15 fp8 DoubleRow + 31 bf16 per
        # (chunk, l-half).  Padded-p chunks, M<=128 stationary.
        def emit_chunk_j(p0, M, j, ps_j):
            src8 = a8 if j == 0 else b8
            src16 = a16 if j == 0 else b16
            first = True
            for woff, wstep, moff, mstep in FP8_PAIRS:
                lhsT = pair_ap(w8, woff + p0, wstep, M)
                rhs = pair_ap(src8, moff, mstep, NL)
                nc.tensor.matmul(
                    ps_j[0:M], lhsT, rhs, start=first, stop=False, perf_mode=DR
                )
                first = False
            for woff, moff, kp in BF16_PLANES:
                lhsT = w16[0:kp, woff + p0 : woff + p0 + M]
                rhs = src16[0:kp, moff : moff + NL]
                last = kp == C
                nc.tensor.matmul(ps_j[0:M], lhsT, rhs, start=False, stop=last)

        def alloc_ps(ci):
            return [
                psum.tile([128, NL], f32, tag="ps", name=f"ps_{ci}_{j}")
                for j in range(2)
            ]

        def emit_epi_j0(ci, M, ps):
            sc0 = scr.tile([128, NL], f32, tag="sc", name=f"sc0_{ci}")
            red0 = reds.tile([128, 1], f32, tag="red", name=f"red0_{ci}")
            nc.vector.tensor_tensor(
                out=sc0[0:M], in0=ps[0][0:M], in1=inv_bc[0:M, 0:NL], op=MULT
            )
            nc.vector.tensor_reduce(
                out=red0[0:M], in_=sc0[0:M], axis=mybir.AxisListType.X, op=MAX
            )
            return red0

        def emit_epi_j1(ci, M, ps, red0):
            sc1 = scr.tile([128, NL], f32, tag="sc", name=f"sc1_{ci}")
            red1 = reds.tile([128, 1], f32, tag="red2", name=f"red1_{ci}")
            nc.vector.tensor_tensor(
                out=sc1[0:M], in0=ps[1][0:M], in1=inv_bc[0:M, NL : 2 * NL], op=MULT
            )
            nc.vector.tensor_reduce(
                out=red1[0:M], in_=sc1[0:M], axis=mybir.AxisListType.X, op=MAX
            )
            nc.vector.tensor_tensor(
                out=red_all[0:M, ci : ci + 1], in0=red0[0:M], in1=red1[0:M], op=MAX
            )

        chunk_ps = {}
        chunk_red = {}

        # chunk 0, both l-halves
        chunk_ps[0] = alloc_ps(0)
        emit_chunk_j(*P_CHUNKS[0], 0, chunk_ps[0][0])
        emit_chunk_j(*P_CHUNKS[0], 1, chunk_ps[0][1])

        # norm matmuls: f16 ones channel-sum per l-half, placed after chunk
        # 0 so the fp16 tree has a wide completion margin.
        nm = [psum.tile([1, NL], f32, tag="ps", name=f"nm_{j}") for j in range(2)]
        nc.tensor.matmul(nm[0][:], ones_col[0:C, :], normc_f[0:C, :], start=True, stop=True)
        nc.tensor.matmul(nm[1][:], ones_col[C : 2 * C, :], normc_f[C : 2 * C, :], start=True, stop=True)

        nsq = nrm.tile([1, 2 * NL], f32)
        nc.vector.tensor_scalar_max(nsq[:, 0:NL], nm[0][:], 1e-8)
        nc.vector.tensor_scalar_max(nsq[:, NL : 2 * NL], nm[1][:], 1e-8)
        nrm_s = nrm.tile([1, 2 * NL], f32)
        nc.scalar.activation(nrm_s[:], nsq[:], SQRT)
        inv_s = nrm.tile([1, 2 * NL], f32)
        nc.vector.reciprocal(inv_s[:], nrm_s[:])
        inv16 = nrm.tile([1, 2 * NL], f16)
        nc.vector.tensor_copy(inv16[:], inv_s[:])

        # chunk 1 first half
        chunk_ps[1] = alloc_ps(1)
        emit_chunk_j(*P_CHUNKS[1], 0, chunk_ps[1][0])

        # rank-1 broadcast of 1/norm to all 128 partitions, between chunk
        # 1's halves (inv16 is ready by the time the PE arrives here).
        inv_bc = nrm.tile([128, 2 * NL], f32)
        for j in range(2):
            ip = psum.tile([128, NL], f32, tag="ps", name=f"ip_{j}")
            nc.tensor.matmul(ip[:], ones_row[:], inv16[:, NL * j : NL * (j + 1)], start=True, stop=True)
            nc.vector.tensor_copy(inv_bc[:, NL * j : NL * (j + 1)], ip[:])

        emit_chunk_j(*P_CHUNKS[1], 1, chunk_ps[1][1])

        chunk_red[0] = emit_epi_j0(0, P_CHUNKS[0][1], chunk_ps[0])
        chunk_ps[2] = alloc_ps(2)
        emit_chunk_j(*P_CHUNKS[2], 0, chunk_ps[2][0])
        emit_epi_j1(0, P_CHUNKS[0][1], chunk_ps[0], chunk_red[0])
        nc.gpsimd.dma_start(out_d[:, 0:1], red_all[:, 0:1])
        chunk_red[1] = emit_epi_j0(1, P_CHUNKS[1][1], chunk_ps[1])
        emit_chunk_j(*P_CHUNKS[2], 1, chunk_ps[2][1])
        emit_epi_j1(1, P_CHUNKS[1][1], chunk_ps[1], chunk_red[1])
        nc.gpsimd.dma_start(out_d[:, 1:2], red_all[:, 1:2])
        chunk_ps[3] = alloc_ps(3)
        emit_chunk_j(*P_CHUNKS[3], 0, chunk_ps[3][0])
        chunk_red[2] = emit_epi_j0(2, P_CHUNKS[2][1], chunk_ps[2])
        emit_chunk_j(*P_CHUNKS[3], 1, chunk_ps[3][1])
        emit_epi_j1(2, P_CHUNKS[2][1], chunk_ps[2], chunk_red[2])
        nc.gpsimd.dma_start(out_d[:, 2:3], red_all[:, 2:3])
        chunk_red[3] = emit_epi_j0(3, P_CHUNKS[3][1], chunk_ps[3])
        emit_epi_j1(3, P_CHUNKS[3][1], chunk_ps[3], chunk_red[3])
        nc.gpsimd.dma_start(out_d[:, 3:4], red_all[:, 3:4])

    nc.compile()
    return nc


def _get_program():
    global _PROGRAM
    if _PROGRAM is None:
        _PROGRAM = _build_program()
    return _PROGRAM


F8 = ml_dtypes.float8_e4m3
BF16 = ml_dtypes.bfloat16


def _stack_shift(lo, hi):
    """[64, R, 30] + [64, R, 30] -> [128, R, 30] f32."""
    return np.concatenate([lo, hi], axis=0)


def _pad32(t):
    """[128, R, 30] -> [128, R*32] f32, rows padded 30->32 with 0."""
    padded = np.zeros((128, IM2_ROWS, WPAD), np.float32)
    padded[:, :, :WP] = t
    return padded.reshape(128, -1)


def make_in_maps(im1: np.ndarray, im2: np.ndarray):
    im1 = np.asarray(im1, dtype=np.float32)
    im2 = np.asarray(im2, dtype=np.float32)
    in_maps = []
    for b in range(B):
        i1 = im1[b]
        i1pad = np.concatenate([i1, np.zeros((C, 1, W), np.float32)], axis=1)
        mov = {0: [], 1: []}  # per l-half list of [128, R, 30] f32 tiles
        for bi, dx in enumerate(DX_BASES):
            for half in (0, 1):
                y0 = HALF * half
                r = i1[:, y0 : y0 + IM2_ROWS, :]
                rs = i1pad[:, y0 + 1 : y0 + 1 + IM2_ROWS, :]
                if dx < 10:
                    mov[half].append(
                        _stack_shift(r[:, :, dx : dx + WP], r[:, :, dx + 1 : dx + WP + 1])
                    )
                else:
                    mov[half].append(_stack_shift(r[:, :, 10:40], rs[:, :, 10:40]))
        def mcat(tiles, lo, hi, dt_):
            return np.ascontiguousarray(
                np.concatenate(
                    [t.reshape(128, -1) for t in tiles[lo:hi]], axis=1
                ).astype(dt_)
            )
        a8 = mcat(mov[0], 0, 3, F8)
        b8 = mcat(mov[1], 0, 3, F8)
        a16 = mcat(mov[0], 2, 6, BF16)
        b16 = mcat(mov[1], 2, 6, BF16)
        im1n = np.ascontiguousarray(
            np.concatenate(
                [i1[:, 0:IM2_ROWS, :], i1[:, HALF : HALF + IM2_ROWS, :]], axis=0
            ).astype(BF16)
        )
        for h in range(2):
            y0 = HALF * h
            i2 = im2[b][:, y0 : y0 + IM2_ROWS, :]
            i2pad = np.concatenate(
                [im2[b], np.zeros((C, 1, W), np.float32)], axis=1
            )[:, y0 + 1 : y0 + 1 + IM2_ROWS, :]
            wt = []
            for bi, dx in enumerate(DX_BASES):
                if dx < 10:
                    wt.append(
                        _pad32(_stack_shift(i2[:, :, dx : dx + WP], i2[:, :, dx + 1 : dx + WP + 1]))
                    )
                else:
                    wt.append(_pad32(_stack_shift(i2[:, :, 10:40], i2pad[:, :, 10:40])))
            m = {
                "a8": a8,
                "b8": b8,
                "a16": a16,
                "b16": b16,
                "im1n": im1n,
                "w8": np.ascontiguousarray(np.concatenate(wt[0:3], axis=1).astype(F8)),
                "w16": np.ascontiguousarray(np.concatenate(wt[2:6], axis=1).astype(BF16)),
            }
            in_maps.append(m)
    return in_maps


def _half_from_cols(cols):
    flat = np.empty((HALF * WPAD,), dtype=np.float32)
    for ci, (p0, M) in enumerate(P_CHUNKS):
        flat[p0 : p0 + M] = cols[0:M, ci]
    return flat.reshape(HALF, WPAD)[:, :WP]


def assemble(results):
    out = np.empty((B, 1, HP, WP), dtype=np.float32)
    for b in range(B):
        top = _half_from_cols(results[2 * b]["out"])
        bot = _half_from_cols(results[2 * b + 1]["out"])
        out[b, 0] = np.concatenate([top, bot], axis=0)
    return out


def run(im1: np.ndarray, im2: np.ndarray, trace: bool = False):
    from concourse import bass_utils

    nc = _get_program()
    res = bass_utils.run_bass_kernel_spmd(
        nc, make_in_maps(im1, im2), core_ids=list(range(N_CORES)), trace=trace
    )
    return assemble(res.results), res


def kernel(im1: np.ndarray, im2: np.ndarray) -> np.ndarray:
    out, _ = run(np.asarray(im1), np.asarray(im2))
    return out
